# revision 4
# baseline (speedup 1.0000x reference)
"""Trainium2 Bass kernel for nn_Loss_40510131536268.

Algorithm
---------
The reference is a T-step normalized forward recursion over a fixed sparse
graph (E=16384 edges on V=2048 nodes), batched over B=32:

    log_C   = logsumexp(log_prev over out-nodes)
    prop    = exp(log_prev[:, out_idxs] - log_C)
    combined= scatter_add(prop -> in_idxs)
    log_curr= log_safe(combined) + x_t
    result  = log(sum over end nodes of exp(log_curr)) + sum(log_C)  at t+1==len

In probability space the per-step normalization by C cancels exactly in the
final result, so the recursion linearizes to

    U_t = (U_{t-1} @ A) * X_t        A[u,w] = #edges u->w,  X_t = exp(x_t)

with result[b] = log( sum_v U_{L-1}[b,v] * end_w[v] ) plus exact bookkeeping
for the per-step scales folded into X to keep bf16 in range.  The EPS clamps
of the reference only affect mass at relative level e^-64 — invisible here.

Device schedule (per core, data-parallel over B: 4 batch rows per core)
-----------------------------------------------------------------------
Per step:
  - 64 accumulating matmuls [K=128, M=4, N=512] issued k-outer/group-inner
    with tile_position=(0, 32g): the PE runs 4 column-group streams
    concurrently (one PSUM bank; group g owns partitions 32g..32g+3), which
    runs at ~4x the serial stream rate and is the dominant cost (~3.4us of
    A-matrix streaming per step is the hardware floor).
  - 16 DVE multiplies (per group/k-tile piece): un = PSUM * X_t, bf16.
  - 16 PE transposes [4,128] -> [128,4] (vs a per-group replicated identity)
    plus 16 DVE copies rebuild the v-major state tiles for the next step.
    The first piece unblocks the next step's first matmul after ~3 engine
    hops; the rest arrive just in time behind it.
  - DMAs are batched 8 steps at a time (4 X loads + 1 u_hist store per
    block) because every DMA instruction costs ~0.6us on the shared HWDGE.
Host: exp/scaling prep, final gather of E_t = U_t . end_w at t = L_b - 1.
No collectives; 8 cores each run an independent batch shard.
"""

import numpy as np
import ml_dtypes

bf16 = ml_dtypes.bfloat16

V, B, T, E, S = 2048, 32, 256, 16384, 128
NCORES = 8
BL = B // NCORES        # 4 batch rows per core
NK = V // 128           # 16 contraction tiles
NG = 4                  # column-tile groups / output chunks of 512
XBATCH = 8              # steps per DMA batch
EPS = float(np.exp(-64.0))

_PROGRAM_CACHE = {}


def _split_multi_waits(nc):
    """walrus in this toolchain rejects compute instructions carrying more
    than one semaphore wait ("Too many sync wait commands").  Split extra
    waits onto no-op instructions inserted immediately before, on the same
    engine (engine-local program order preserves the gating semantics)."""
    import concourse.mybir as mybir

    skip = (
        mybir.InstCall,
        mybir.InstUnconditionalBranch,
        mybir.InstCompareAndBranch,
        mybir.InstIndirectBranch,
        mybir.InstHalt,
    )
    for f in nc.m.functions:
        for blk in f.blocks:
            out = []
            changed = False
            for inst in blk.instructions:
                si = inst.sync_info
                if (
                    si is not None
                    and si.on_wait
                    and len(si.on_wait) > 1
                    and not isinstance(inst, skip)
                ):
                    waits = list(si.on_wait)
                    for w in waits[:-1]:
                        out.append(
                            mybir.InstNoOp(
                                name=nc.get_next_instruction_name(),
                                engine=inst.engine,
                                ins=[],
                                outs=[],
                                bass_nofuse=True,
                                sync_info=mybir.SyncInfo(on_wait=[w], on_update=[]),
                            )
                        )
                    inst.sync_info = mybir.SyncInfo(
                        on_wait=[waits[-1]], on_update=list(si.on_update or [])
                    )
                    changed = True
                out.append(inst)
            if changed:
                blk.instructions = out


def build_program(n_steps, split_waits=True, outer_reps=1):
    """Build the SPMD Bass/Tile program (identical on all 8 cores).

    outer_reps: wrap the step loop in a hardware For_i (timing use only)."""
    import concourse.bass as bass
    import concourse.mybir as mybir
    from concourse.tile import TileContext

    f32 = mybir.dt.float32
    b16 = mybir.dt.bfloat16

    nc = bass.Bass()
    a_in = nc.declare_dram_parameter("a_rhs", [128, NK * V], b16, isOutput=False)
    u0_in = nc.declare_dram_parameter("u0t", [128, NK * BL], b16, isOutput=False)
    xs_in = nc.declare_dram_parameter("xs", [n_steps * BL, V], b16, isOutput=False)
    id_in = nc.declare_dram_parameter("ident", [128, BL], b16, isOutput=False)
    uh_out = nc.declare_dram_parameter(
        "u_hist", [n_steps * 128, NK * BL], b16, isOutput=True
    )

    nb = (n_steps + XBATCH - 1) // XBATCH

    with TileContext(nc) as tc:
        with (
            tc.tile_pool(name="const", bufs=1) as cpool,
            tc.tile_pool(name="xv", bufs=2) as xpool,
            tc.tile_pool(name="ur", bufs=2) as urpool,
            tc.tile_pool(name="unp", bufs=2) as unpool,
            tc.tile_pool(name="pr", bufs=2, space="PSUM") as prpool,
            tc.tile_pool(name="pt", bufs=2, space="PSUM") as ptpool,
        ):
            a_sb = cpool.tile([128, NK * V], b16, tag="a_sb")
            nc.gpsimd.dma_start(a_sb[:, :], a_in[:, :])
            # identity [4,4] replicated at partitions 32g..32g+3 per group
            ident = cpool.tile([128, BL], b16, tag="ident")
            nc.gpsimd.dma_start(ident[:, :], id_in[:, :])
            u_init = cpool.tile([128, NK * BL], b16, tag="u_init")
            nc.gpsimd.dma_start(u_init[:, :], u0_in[:, :])

            state = {}

            def lhsT(i, k):
                if i == 0:
                    return u_init[:, k * BL : (k + 1) * BL]
                tile, _ = state[(i - 1) % 2]
                s = (i - 1) % XBATCH
                return tile[:, s * 64 + k * BL : s * 64 + (k + 1) * BL]

            u_roll = [
                urpool.tile([128, XBATCH * 64], b16, tag="u_roll", name=f"u_roll{p}")
                for p in range(2)
            ]

            def body(blk):
                xr = xpool.tile([128, XBATCH * 512], b16, tag="xr")
                i0 = blk * XBATCH
                nsteps_blk = min(XBATCH, n_steps - i0)
                # dst[32g+b, s*512+c] = xs[(i0+s)*BL + b, 512g + c]
                for g in range(NG):
                    nc.sync.dma_start(
                        xr[32 * g : 32 * g + BL, 0 : nsteps_blk * 512],
                        xs_in[i0 * BL : (i0 + nsteps_blk) * BL, :]
                        .rearrange("(s b) (g c) -> b g s c", b=BL, g=4)[:, g, :, :],
                    )
                ur = u_roll[blk % 2]
                for s in range(nsteps_blk):
                    i = i0 + s
                    P = prpool.tile([128, 512], f32, tag="P")
                    for k in range(NK):
                        for g in range(NG):
                            nc.tensor.matmul(
                                P[32 * g : 32 * g + BL, :],
                                lhsT(i, k),
                                a_sb[:, k * V + g * 512 : k * V + (g + 1) * 512],
                                start=(k == 0),
                                stop=(k == NK - 1),
                                tile_position=(0, 32 * g),
                            )
                    un = unpool.tile([128, 512], b16, tag="un")
                    tp = ptpool.tile([128, 64], b16, tag="tp")
                    state[i % 2] = (ur, s * 64)
                    for g in range(NG):
                        for kk in range(4):
                            k = 4 * g + kk
                            nc.vector.tensor_mul(
                                un[32 * g : 32 * g + BL, kk * 128 : (kk + 1) * 128],
                                P[32 * g : 32 * g + BL, kk * 128 : (kk + 1) * 128],
                                xr[
                                    32 * g : 32 * g + BL,
                                    s * 512 + kk * 128 : s * 512 + (kk + 1) * 128,
                                ],
                            )
                            nc.tensor.transpose(
                                tp[:, k * BL : (k + 1) * BL],
                                un[32 * g : 32 * g + BL, kk * 128 : (kk + 1) * 128],
                                ident[32 * g : 32 * g + BL, :],
                                tile_position=(32 * g, 0),
                            )
                            nc.vector.tensor_copy(
                                ur[:, s * 64 + k * BL : s * 64 + (k + 1) * BL],
                                tp[:, k * BL : (k + 1) * BL],
                            )
                nc.scalar.dma_start(
                    uh_out[i0 * 128 : (i0 + nsteps_blk) * 128, :]
                    .rearrange("(s p) c -> p s c", s=nsteps_blk),
                    ur[:, 0 : nsteps_blk * 64]
                    .rearrange("p (s c) -> p s c", s=nsteps_blk),
                )

            if outer_reps == 1:
                for blk in range(nb):
                    body(blk)
            else:
                with tc.For_i(0, outer_reps):
                    for blk in range(nb):
                        body(blk)
    if split_waits:
        _split_multi_waits(nc)
    return nc


def _prep_host(inputs, n_steps):
    """Host-side preprocessing shared by all cores."""
    x = np.asarray(inputs["extracted_log_probs"], np.float32)   # [V,B,T]
    in_idxs = np.asarray(inputs["in_idxs"]).astype(np.int64)
    out_idxs = np.asarray(inputs["out_idxs"]).astype(np.int64)
    start_idxs = np.asarray(inputs["start_idxs"]).astype(np.int64)
    end_idxs = np.asarray(inputs["end_idxs"]).astype(np.int64)

    xt = np.ascontiguousarray(np.transpose(x, (2, 1, 0)))       # [T,B,V]

    A_cnt = np.zeros((V, V), np.float32)
    np.add.at(A_cnt, (out_idxs, in_idxs), 1.0)

    end_w = np.zeros((V,), np.float32)
    np.add.at(end_w, end_idxs, 1.0)

    start_mask = np.zeros((V,), bool)
    start_mask[start_idxs] = True

    # A tiles for the rhs: a_sb[p, k*V + w] = A_cnt[k*128+p, w]
    a_sb = np.ascontiguousarray(
        A_cnt.reshape(NK, 128, V).transpose(1, 0, 2).reshape(128, NK * V)
    ).astype(bf16)

    # U_0 = exp(log_curr0)
    X0 = np.exp(xt[0])                                           # [B,V]
    U0 = np.where(start_mask[None, :], X0, np.float32(EPS)).astype(np.float32)
    U0_16 = U0.astype(bf16)

    # scales sigma_t[b] folded into X' (t = 1..n_steps)
    Xall = np.exp(xt[1 : n_steps + 1])                           # [n,B,V]
    m = Xall.mean(axis=2)                                        # [n,B]
    sigma = (1.0 / (8.0 * m)).astype(np.float32)
    cumlog = np.cumsum(np.log(sigma.astype(np.float64)), axis=0) # [n,B]
    Xs16 = (Xall * sigma[:, :, None]).astype(bf16)               # [n,B,V]

    return dict(a_sb=a_sb, U0_16=U0_16, Xs16=Xs16, cumlog=cumlog, end_w=end_w)


def _core_inputs(prep, core, n_steps):
    bsl = slice(core * BL, (core + 1) * BL)
    # u0t[p, k*BL+b] = U0[b, k*128+p]
    u0c = prep["U0_16"][bsl]                                     # [BL, V]
    u0t = np.ascontiguousarray(
        u0c.reshape(BL, NK, 128).transpose(2, 1, 0).reshape(128, NK * BL)
    )
    xs = np.ascontiguousarray(prep["Xs16"][:, bsl, :].reshape(n_steps * BL, V))
    ident = np.zeros((128, BL), bf16)
    for g in range(NG):
        ident[32 * g : 32 * g + BL, :] = np.eye(BL, dtype=bf16)
    return {"a_rhs": prep["a_sb"], "u0t": u0t, "xs": xs, "ident": ident}


def _postprocess(prep, results, target_lengths, n_steps):
    """results: list of per-core out_maps with 'u_hist'."""
    end_w_kp = prep["end_w"].reshape(NK, 128)                    # [k, p]
    E_dev = np.zeros((n_steps + 1, B), np.float64)
    # t = 0 from host U0 (bf16-rounded, same as device state precision)
    E_dev[0] = prep["U0_16"].astype(np.float32) @ prep["end_w"]
    for c in range(NCORES):
        uh = np.asarray(results[c]["u_hist"]).reshape(n_steps, 128, NK, BL)
        # E[t, b] = sum_{k,p} uh[t, p, k, b] * end_w[k*128+p]
        Ec = np.einsum("tpkb,kp->tb", uh.astype(np.float32), end_w_kp)
        E_dev[1:, c * BL : (c + 1) * BL] = Ec
    lengths = np.asarray(target_lengths).astype(np.int64)
    res = np.zeros((B,), np.float64)
    for b in range(B):
        L = int(lengths[b])
        corr = prep["cumlog"][L - 2, b] if L >= 2 else 0.0
        res[b] = np.log(E_dev[L - 1, b]) - corr
    return (-res).astype(np.float32)


def run_on_device(nc, core_maps, **kwargs):
    from concourse.bass_utils import run_bass_kernel_spmd

    return run_bass_kernel_spmd(nc, core_maps, core_ids=list(range(NCORES)), **kwargs)


def kernel(**inputs) -> np.ndarray:
    lengths = np.asarray(inputs["target_lengths"]).astype(np.int64)
    n_steps = max(1, int(lengths.max()) - 1)
    prep = _prep_host(inputs, n_steps)
    core_maps = [_core_inputs(prep, c, n_steps) for c in range(NCORES)]
    last_err = None
    for attempt in range(3):
        try:
            if n_steps not in _PROGRAM_CACHE:
                _PROGRAM_CACHE[n_steps] = build_program(n_steps)
            nc = _PROGRAM_CACHE[n_steps]
            out = run_on_device(nc, core_maps)
            break
        except Exception as e:                      # flaky axon compile path
            last_err = e
            _PROGRAM_CACHE.pop(n_steps, None)
    else:
        raise last_err
    return _postprocess(prep, out.results, inputs["target_lengths"], n_steps)


# revision 5
# speedup vs baseline: 3.5337x; 3.5337x over previous
"""Trainium2 Bass kernel for nn_Loss_40510131536268.

Algorithm
---------
The reference is a T-step normalized forward recursion over a fixed sparse
graph (E=16384 edges on V=2048 nodes), batched over B=32:

    log_C   = logsumexp(log_prev over out-nodes)
    prop    = exp(log_prev[:, out_idxs] - log_C)
    combined= scatter_add(prop -> in_idxs)
    log_curr= log_safe(combined) + x_t
    result  = log(sum over end nodes of exp(log_curr)) + sum(log_C)  at t+1==len

In probability space the per-step normalization by C cancels exactly in the
final result, so the recursion linearizes to

    U_t = (U_{t-1} @ A) * X_t        A[u,w] = #edges u->w,  X_t = exp(x_t)

with result[b] = log( sum_v U_{L-1}[b,v] * end_w[v] ) plus exact bookkeeping
for the per-step scales folded into X to keep bf16 in range.  The EPS clamps
of the reference only affect mass at relative level e^-64 — invisible here.

Device schedule (per core, data-parallel over B: 4 batch rows per core)
-----------------------------------------------------------------------
Per step:
  - 64 accumulating matmuls [K=128, M=4, N=512] issued k-outer/group-inner
    with tile_position=(0, 32g): the PE runs 4 column-group streams
    concurrently (one PSUM bank; group g owns partitions 32g..32g+3), which
    runs at ~4x the serial stream rate and is the dominant cost (~3.4us of
    A-matrix streaming per step is the hardware floor).
  - 16 DVE multiplies (per group/k-tile piece): un = PSUM * X_t, bf16.
  - 16 PE transposes [4,128] -> [128,4] (vs a per-group replicated identity)
    plus 16 DVE copies rebuild the v-major state tiles for the next step.
    The first piece unblocks the next step's first matmul after ~3 engine
    hops; the rest arrive just in time behind it.
  - DMAs are batched 8 steps at a time (4 X loads + 1 u_hist store per
    block) because every DMA instruction costs ~0.6us on the shared HWDGE.
Host: exp/scaling prep, final gather of E_t = U_t . end_w at t = L_b - 1.
No collectives; 8 cores each run an independent batch shard.
"""

import numpy as np
import ml_dtypes

bf16 = ml_dtypes.bfloat16

V, B, T, E, S = 2048, 32, 256, 16384, 128
NCORES = 8
BL = B // NCORES        # 4 batch rows per core
NK = V // 128           # 16 contraction tiles
NG = 4                  # column-tile groups / output chunks of 512
XBATCH = 8              # steps per DMA batch
EPS = float(np.exp(-64.0))

_PROGRAM_CACHE = {}


def _split_multi_waits(nc):
    """walrus in this toolchain rejects compute instructions carrying more
    than one semaphore wait ("Too many sync wait commands").  Split extra
    waits onto no-op instructions inserted immediately before, on the same
    engine (engine-local program order preserves the gating semantics)."""
    import concourse.mybir as mybir

    skip = (
        mybir.InstCall,
        mybir.InstUnconditionalBranch,
        mybir.InstCompareAndBranch,
        mybir.InstIndirectBranch,
        mybir.InstHalt,
    )
    for f in nc.m.functions:
        for blk in f.blocks:
            out = []
            changed = False
            for inst in blk.instructions:
                si = inst.sync_info
                if (
                    si is not None
                    and si.on_wait
                    and len(si.on_wait) > 1
                    and not isinstance(inst, skip)
                ):
                    waits = list(si.on_wait)
                    for w in waits[:-1]:
                        out.append(
                            mybir.InstNoOp(
                                name=nc.get_next_instruction_name(),
                                engine=inst.engine,
                                ins=[],
                                outs=[],
                                bass_nofuse=True,
                                sync_info=mybir.SyncInfo(on_wait=[w], on_update=[]),
                            )
                        )
                    inst.sync_info = mybir.SyncInfo(
                        on_wait=[waits[-1]], on_update=list(si.on_update or [])
                    )
                    changed = True
                out.append(inst)
            if changed:
                blk.instructions = out


def build_program(n_steps, split_waits=True, outer_reps=1):
    """Build the SPMD Bass/Tile program (identical on all 8 cores).

    outer_reps: wrap the step loop in a hardware For_i (timing use only)."""
    import concourse.bass as bass
    import concourse.mybir as mybir
    from concourse.tile import TileContext

    f32 = mybir.dt.float32
    b16 = mybir.dt.bfloat16

    nc = bass.Bass()
    a_in = nc.declare_dram_parameter("a_rhs", [128, NK * V], b16, isOutput=False)
    u0_in = nc.declare_dram_parameter("u0t", [128, NK * BL], b16, isOutput=False)
    xs_in = nc.declare_dram_parameter("xs", [n_steps * BL, V], b16, isOutput=False)
    id_in = nc.declare_dram_parameter("ident", [128, BL], b16, isOutput=False)
    uh_out = nc.declare_dram_parameter(
        "u_hist", [n_steps * 128, NK * BL], b16, isOutput=True
    )

    nb = (n_steps + XBATCH - 1) // XBATCH

    with TileContext(nc) as tc:
        with (
            tc.tile_pool(name="const", bufs=1) as cpool,
            tc.tile_pool(name="xv", bufs=2) as xpool,
            tc.tile_pool(name="ur", bufs=2) as urpool,
            tc.tile_pool(name="unp", bufs=3) as unpool,
            tc.tile_pool(name="pr", bufs=3, space="PSUM") as prpool,
            tc.tile_pool(name="pt", bufs=2, space="PSUM") as ptpool,
        ):
            a_sb = cpool.tile([128, NK * V], b16, tag="a_sb")
            nc.gpsimd.dma_start(a_sb[:, :], a_in[:, :])
            # identity [4,4] replicated at partitions 32g..32g+3 per group
            ident = cpool.tile([128, BL], b16, tag="ident")
            nc.gpsimd.dma_start(ident[:, :], id_in[:, :])
            u_init = cpool.tile([128, NK * BL], b16, tag="u_init")
            nc.gpsimd.dma_start(u_init[:, :], u0_in[:, :])

            state = {}

            def lhsT(i, k):
                if i == 0:
                    return u_init[:, k * BL : (k + 1) * BL]
                tile, _ = state[(i - 1) % 2]
                s = (i - 1) % XBATCH
                return tile[:, s * 64 + k * BL : s * 64 + (k + 1) * BL]

            u_roll = [
                urpool.tile([128, XBATCH * 64], b16, tag="u_roll", name=f"u_roll{p}")
                for p in range(2)
            ]

            def body(blk):
                xr = xpool.tile([128, XBATCH * 512], b16, tag="xr")
                i0 = blk * XBATCH
                nsteps_blk = min(XBATCH, n_steps - i0)
                # dst[32g+b, s*512+c] = xs[(i0+s)*BL + b, 512g + c]
                for g in range(NG):
                    nc.sync.dma_start(
                        xr[32 * g : 32 * g + BL, 0 : nsteps_blk * 512],
                        xs_in[i0 * BL : (i0 + nsteps_blk) * BL, :]
                        .rearrange("(s b) (g c) -> b g s c", b=BL, g=4)[:, g, :, :],
                    )
                ur = u_roll[blk % 2]
                for s in range(nsteps_blk):
                    i = i0 + s
                    P = prpool.tile([128, 512], f32, tag="P")
                    for k in range(NK):
                        for g in range(NG):
                            nc.tensor.matmul(
                                P[32 * g : 32 * g + BL, :],
                                lhsT(i, k),
                                a_sb[:, k * V + g * 512 : k * V + (g + 1) * 512],
                                start=(k == 0),
                                stop=(k == NK - 1),
                                tile_position=(0, 32 * g),
                            )
                    un = unpool.tile([128, 512], b16, tag="un")
                    tp = ptpool.tile([128, 64], b16, tag="tp")
                    state[i % 2] = (ur, s * 64)
                    for g in range(NG):
                        for kk in range(4):
                            k = 4 * g + kk
                            nc.vector.tensor_mul(
                                un[32 * g : 32 * g + BL, kk * 128 : (kk + 1) * 128],
                                P[32 * g : 32 * g + BL, kk * 128 : (kk + 1) * 128],
                                xr[
                                    32 * g : 32 * g + BL,
                                    s * 512 + kk * 128 : s * 512 + (kk + 1) * 128,
                                ],
                            )
                            nc.tensor.transpose(
                                tp[:, k * BL : (k + 1) * BL],
                                un[32 * g : 32 * g + BL, kk * 128 : (kk + 1) * 128],
                                ident[32 * g : 32 * g + BL, :],
                                tile_position=(32 * g, 0),
                            )
                            nc.vector.tensor_copy(
                                ur[:, s * 64 + k * BL : s * 64 + (k + 1) * BL],
                                tp[:, k * BL : (k + 1) * BL],
                            )
                nc.scalar.dma_start(
                    uh_out[i0 * 128 : (i0 + nsteps_blk) * 128, :]
                    .rearrange("(s p) c -> p s c", s=nsteps_blk),
                    ur[:, 0 : nsteps_blk * 64]
                    .rearrange("p (s c) -> p s c", s=nsteps_blk),
                )

            if outer_reps == 1:
                for blk in range(nb):
                    body(blk)
            else:
                with tc.For_i(0, outer_reps):
                    for blk in range(nb):
                        body(blk)
    if split_waits:
        _split_multi_waits(nc)
    return nc


def _prep_host(inputs, n_steps):
    """Host-side preprocessing shared by all cores."""
    x = np.asarray(inputs["extracted_log_probs"], np.float32)   # [V,B,T]
    in_idxs = np.asarray(inputs["in_idxs"]).astype(np.int64)
    out_idxs = np.asarray(inputs["out_idxs"]).astype(np.int64)
    start_idxs = np.asarray(inputs["start_idxs"]).astype(np.int64)
    end_idxs = np.asarray(inputs["end_idxs"]).astype(np.int64)

    xt = np.ascontiguousarray(np.transpose(x, (2, 1, 0)))       # [T,B,V]

    A_cnt = np.zeros((V, V), np.float32)
    np.add.at(A_cnt, (out_idxs, in_idxs), 1.0)

    end_w = np.zeros((V,), np.float32)
    np.add.at(end_w, end_idxs, 1.0)

    start_mask = np.zeros((V,), bool)
    start_mask[start_idxs] = True

    # A tiles for the rhs: a_sb[p, k*V + w] = A_cnt[k*128+p, w]
    a_sb = np.ascontiguousarray(
        A_cnt.reshape(NK, 128, V).transpose(1, 0, 2).reshape(128, NK * V)
    ).astype(bf16)

    # U_0 = exp(log_curr0)
    X0 = np.exp(xt[0])                                           # [B,V]
    U0 = np.where(start_mask[None, :], X0, np.float32(EPS)).astype(np.float32)
    U0_16 = U0.astype(bf16)

    # scales sigma_t[b] folded into X' (t = 1..n_steps)
    Xall = np.exp(xt[1 : n_steps + 1])                           # [n,B,V]
    m = Xall.mean(axis=2)                                        # [n,B]
    sigma = (1.0 / (8.0 * m)).astype(np.float32)
    cumlog = np.cumsum(np.log(sigma.astype(np.float64)), axis=0) # [n,B]
    Xs16 = (Xall * sigma[:, :, None]).astype(bf16)               # [n,B,V]

    return dict(a_sb=a_sb, U0_16=U0_16, Xs16=Xs16, cumlog=cumlog, end_w=end_w)


def _core_inputs(prep, core, n_steps):
    bsl = slice(core * BL, (core + 1) * BL)
    # u0t[p, k*BL+b] = U0[b, k*128+p]
    u0c = prep["U0_16"][bsl]                                     # [BL, V]
    u0t = np.ascontiguousarray(
        u0c.reshape(BL, NK, 128).transpose(2, 1, 0).reshape(128, NK * BL)
    )
    xs = np.ascontiguousarray(prep["Xs16"][:, bsl, :].reshape(n_steps * BL, V))
    ident = np.zeros((128, BL), bf16)
    for g in range(NG):
        ident[32 * g : 32 * g + BL, :] = np.eye(BL, dtype=bf16)
    return {"a_rhs": prep["a_sb"], "u0t": u0t, "xs": xs, "ident": ident}


def _postprocess(prep, results, target_lengths, n_steps):
    """results: list of per-core out_maps with 'u_hist'."""
    end_w_kp = prep["end_w"].reshape(NK, 128)                    # [k, p]
    E_dev = np.zeros((n_steps + 1, B), np.float64)
    # t = 0 from host U0 (bf16-rounded, same as device state precision)
    E_dev[0] = prep["U0_16"].astype(np.float32) @ prep["end_w"]
    for c in range(NCORES):
        uh = np.asarray(results[c]["u_hist"]).reshape(n_steps, 128, NK, BL)
        # E[t, b] = sum_{k,p} uh[t, p, k, b] * end_w[k*128+p]
        Ec = np.einsum("tpkb,kp->tb", uh.astype(np.float32), end_w_kp)
        E_dev[1:, c * BL : (c + 1) * BL] = Ec
    lengths = np.asarray(target_lengths).astype(np.int64)
    res = np.zeros((B,), np.float64)
    for b in range(B):
        L = int(lengths[b])
        corr = prep["cumlog"][L - 2, b] if L >= 2 else 0.0
        res[b] = np.log(E_dev[L - 1, b]) - corr
    return (-res).astype(np.float32)


def run_on_device(nc, core_maps, **kwargs):
    from concourse.bass_utils import run_bass_kernel_spmd

    return run_bass_kernel_spmd(nc, core_maps, core_ids=list(range(NCORES)), **kwargs)


def kernel(**inputs) -> np.ndarray:
    lengths = np.asarray(inputs["target_lengths"]).astype(np.int64)
    n_steps = max(1, int(lengths.max()) - 1)
    prep = _prep_host(inputs, n_steps)
    core_maps = [_core_inputs(prep, c, n_steps) for c in range(NCORES)]
    last_err = None
    for attempt in range(3):
        try:
            if n_steps not in _PROGRAM_CACHE:
                _PROGRAM_CACHE[n_steps] = build_program(n_steps)
            nc = _PROGRAM_CACHE[n_steps]
            out = run_on_device(nc, core_maps)
            break
        except Exception as e:                      # flaky axon compile path
            last_err = e
            _PROGRAM_CACHE.pop(n_steps, None)
    else:
        raise last_err
    return _postprocess(prep, out.results, inputs["target_lengths"], n_steps)
